# revision 14
# baseline (speedup 1.0000x reference)
"""Distributed causal multi-head attention for 8 TRN2 NeuronCores.

Sharding: tensor parallel over heads (3 per core), data parallel over batch
(cores 0-3 = batch 0, cores 4-7 = batch 1).  The kernel is software-pipelined
over four 512-query chunks: x-load/transpose + q/k/v projection for chunk n+1
are interleaved with attention for chunk n, so the PE array never drains while
ScalarE streams the softmax exps.  v is produced directly in natural [token,
head*64] layout (no re-transpose).  After each chunk's attention, the three
normalized z^T head-slabs are AllGathered within the 4-core batch group
(overlapped with the next chunk's compute); every core then runs the output
projection for its own 128-row quarter of that chunk, so the serial tail is
just the last AllGather plus one small projection slab.
"""
import sys
import math
import numpy as np

sys.path.insert(0, "/opt/trn_rl_repo")

D_MODEL, N_HEADS, D_HEAD = 768, 12, 64
BATCH, SEQ = 2, 2048
HPC = 3              # heads per core
GROUP = 4            # cores per batch group
N_CORES = 8
CHQ = 512            # query-chunk width
KT = 128             # key-tile height
NQC = SEQ // CHQ     # 4
NT = SEQ // KT       # 16 token tiles
ND = D_MODEL // 128  # 6 contraction chunks
SCALE = 1.0 / math.sqrt(D_HEAD)

_BUILT = None
DEBUG_TAPS = False


def _build():
    import concourse.bass as bass
    import concourse.bacc as bacc
    import concourse.mybir as mybir
    import concourse.tile as tile
    from concourse.masks import make_identity

    f32 = mybir.dt.float32
    bf16 = mybir.dt.bfloat16
    FT = mybir.ActivationFunctionType

    nc = bacc.Bacc("TRN2", target_bir_lowering=False, debug=False,
                   num_devices=N_CORES)

    x_d = nc.dram_tensor("x", [SEQ, D_MODEL], f32, kind="ExternalInput")
    wqk_d = nc.dram_tensor("wqk", [D_MODEL, 384], f32, kind="ExternalInput")
    mqk_d = nc.dram_tensor("mqk", [D_MODEL, 384], f32, kind="ExternalInput")
    wv_d = nc.dram_tensor("wv", [D_MODEL, 192], f32, kind="ExternalInput")
    mv_d = nc.dram_tensor("mv", [D_MODEL, 192], f32, kind="ExternalInput")
    wo_d = nc.dram_tensor("wo", [N_HEADS * D_HEAD, D_MODEL], f32, kind="ExternalInput")
    mo_d = nc.dram_tensor("mo", [N_HEADS * D_HEAD, D_MODEL], f32, kind="ExternalInput")
    bqk_d = nc.dram_tensor("bqk", [128, 4], f32, kind="ExternalInput")
    bv_d = nc.dram_tensor("bv", [1, 192], f32, kind="ExternalInput")
    bo_d = nc.dram_tensor("bo", [1, D_MODEL], f32, kind="ExternalInput")
    out_d = nc.dram_tensor("out", [CHQ, D_MODEL], f32, kind="ExternalOutput")
    dbg = {}
    if DEBUG_TAPS:
        for name, shape, dt in [("dbg_xT", [128, SEQ], bf16), ("dbg_qA", [128, SEQ], bf16),
                                ("dbg_vn", [128, 195], bf16), ("dbg_s65", [128, CHQ], f32),
                                ("dbg_rcb", [1, CHQ], bf16), ("dbg_zA", [64, CHQ], bf16),
                                ("dbg_ago", [768, CHQ], bf16), ("dbg_wo", [128, D_MODEL], bf16),
                                ("dbg_P", [128, 2 * CHQ], bf16)]:
            dbg[name] = nc.dram_tensor(name, shape, dt, kind="ExternalOutput")

    with tile.TileContext(nc) as tc:
        with tc.tile_pool(name="const", bufs=1) as constp, \
             tc.tile_pool(name="dram", bufs=1, space="DRAM") as dramp, \
             tc.tile_pool(name="wt", bufs=1) as wtp, \
             tc.tile_pool(name="big", bufs=1) as bigp, \
             tc.tile_pool(name="wld", bufs=2) as wldp, \
             tc.tile_pool(name="xin", bufs=2) as xinp, \
             tc.tile_pool(name="pst", bufs=4) as pstp, \
             tc.tile_pool(name="nrm", bufs=2) as nrmp, \
             tc.tile_pool(name="zsb", bufs=2) as zsbp, \
             tc.tile_pool(name="psPP", bufs=2, space="PSUM") as psPP, \
             tc.tile_pool(name="psZ", bufs=1, space="PSUM") as psZ, \
             tc.tile_pool(name="psM", bufs=1, space="PSUM") as psM:

            # ---- constants ----
            ident32 = constp.tile([128, 128], f32, tag="id32")
            make_identity(nc, ident32[:])
            ident_r = constp.tile([128, 128], bf16, tag="idr")
            nc.vector.tensor_copy(ident_r[:], ident32[:])
            # tri[p, f] = 1.0 if f >= p else 0.0 (inclusive-diagonal upper tri
            # of S^T: key p visible to query f)
            tri = constp.tile([KT, KT], f32, tag="tri")
            nc.gpsimd.memset(tri[:], 1.0)
            nc.gpsimd.affine_select(
                out=tri[:], in_=tri[:], compare_op=mybir.AluOpType.is_ge,
                fill=0.0, base=0, channel_multiplier=-1, pattern=[[1, KT]])
            ones3 = constp.tile([128, HPC], f32, tag="ones3")
            nc.vector.memset(ones3[:], 1.0)
            ones1 = constp.tile([1, 128], f32, tag="ones1")
            nc.vector.memset(ones1[:], 1.0)
            ones_r = constp.tile([1, 128], bf16, tag="ones_r")
            nc.vector.tensor_copy(ones_r[:], ones1[:])
            # preload the exp table set off the critical path
            warm1 = constp.tile([1, 128], f32, tag="warm1")
            nc.scalar.activation(warm1[:], ones1[:], FT.Exp, scale=0.1)
            bias_qk = constp.tile([128, 4], f32, tag="bias_qk")
            nc.gpsimd.dma_start(out=bias_qk[:], in_=bqk_d[:])
            bvrow = constp.tile([1, 192], f32, tag="bvrow")
            nc.gpsimd.dma_start(out=bvrow[:], in_=bv_d[:])
            bvrow_r = constp.tile([1, 192], bf16, tag="bvrow_r")
            nc.vector.tensor_copy(bvrow_r[:], bvrow[:])
            bo32 = constp.tile([1, D_MODEL], f32, tag="bo32")
            nc.gpsimd.dma_start(out=bo32[:], in_=bo_d[:])
            bor = constp.tile([1, D_MODEL], bf16, tag="bor")
            nc.vector.tensor_copy(bor[:], bo32[:])

            # ---- persistent SBUF tensors ----
            # xTa: x^T as 6 d-blocks of [128, 2048] side by side
            xTa = bigp.tile([128, ND * SEQ], bf16, tag="xTa")
            qA = bigp.tile([128, SEQ], bf16, tag="qA")   # [q0 | q1]
            kB = bigp.tile([128, SEQ], bf16, tag="kB")   # [k0 | k1]
            qC = bigp.tile([128, SEQ], bf16, tag="qC")   # [q2 | - ]
            kD = bigp.tile([128, SEQ], bf16, tag="kD")   # [k2 | - ]
            vnat = [bigp.tile([128, 65 * HPC], bf16, tag=f"vn{t}", name=f"vn{t}")
                    for t in range(NT)]
            for t in range(NT):
                vv = vnat[t][:].rearrange("p (h c) -> p h c", c=65)
                nc.vector.tensor_copy(vv[:, :, 64], ones3[:])

            # masked weights (bf16)
            wqk_r = [wtp.tile([128, 384], bf16, tag=f"wqk{d}", name=f"wqk{d}")
                     for d in range(ND)]
            wv_r = [wtp.tile([128, 192], bf16, tag=f"wv{d}", name=f"wv{d}")
                    for d in range(ND)]
            wo_r = [wtp.tile([128, D_MODEL], bf16, tag=f"wo{d}", name=f"wo{d}")
                    for d in range(ND)]
            for d in range(ND):
                w32 = wldp.tile([128, D_MODEL], f32, tag="w32")
                m32 = wldp.tile([128, D_MODEL], f32, tag="m32")
                nc.gpsimd.dma_start(out=w32[:, 0:384], in_=wqk_d[128 * d:128 * (d + 1), :])
                nc.gpsimd.dma_start(out=m32[:, 0:384], in_=mqk_d[128 * d:128 * (d + 1), :])
                nc.gpsimd.dma_start(out=w32[:, 384:576], in_=wv_d[128 * d:128 * (d + 1), :])
                nc.gpsimd.dma_start(out=m32[:, 384:576], in_=mv_d[128 * d:128 * (d + 1), :])
                nc.vector.tensor_mul(wqk_r[d][:], w32[:, 0:384], m32[:, 0:384])
                nc.vector.tensor_mul(wv_r[d][:], w32[:, 384:576], m32[:, 384:576])
            for d in range(ND):
                w32 = wldp.tile([128, D_MODEL], f32, tag="w32")
                m32 = wldp.tile([128, D_MODEL], f32, tag="m32")
                nc.gpsimd.dma_start(out=w32[:], in_=wo_d[128 * d:128 * (d + 1), :])
                nc.gpsimd.dma_start(out=m32[:], in_=mo_d[128 * d:128 * (d + 1), :])
                nc.gpsimd.tensor_mul(wo_r[d][:], w32[:], m32[:])

            # DRAM collective buffers (per q-chunk)
            ag_in = [dramp.tile([HPC * 64, CHQ], bf16, tag=f"agin{qc}",
                                name=f"agin{qc}") for qc in range(NQC)]
            ag_out = [dramp.tile([GROUP * HPC * 64, CHQ], bf16, tag=f"agout{qc}",
                                 name=f"agout{qc}") for qc in range(NQC)]

            rank = nc.sync.partition_id()
            rmod = rank % GROUP                    # group-local rank
            coff = rmod * 128                      # query-column offset for E

            # scores head accessors: (kT tile, qT tile, base partition)
            hacc = [(kB, qA, 0), (kB, qA, 64), (kD, qC, 0)]

            # PE warm-up: harmless matmuls into the zps0 bank while x loads
            zps = [psZ.tile([128, CHQ], f32, tag=f"zps{h}", name=f"zps{h}")
                   for h in range(HPC)]
            for _ in range(8):
                nc.tensor.matmul(zps[0][:, 0:128], ident_r[:], ident_r[:],
                                 start=True, stop=True)

            def setup_unit_A(t):
                # load x token-tile t, cast, transpose into xTa
                xr = xinp.tile([128, D_MODEL], f32, tag="xr", name="xr")
                nc.sync.dma_start(out=xr[:], in_=x_d[KT * t:KT * (t + 1), :])
                xb = xinp.tile([128, D_MODEL], bf16, tag="xb", name="xb")
                nc.vector.tensor_copy(xb[:], xr[:])
                pt = psM.tile([128, D_MODEL], bf16, tag="mp", name="pt")
                for d in range(ND):
                    nc.tensor.transpose(pt[:, 128 * d:128 * (d + 1)],
                                        xb[:, 128 * d:128 * (d + 1)], ident_r[:])
                dst = xTa[:].rearrange("p (d c) -> p d c", d=ND)[:, :, KT * t:KT * (t + 1)]
                src = pt[:].rearrange("p (d c) -> p d c", d=ND)
                nc.vector.tensor_copy(dst, src)

            def setup_unit_B(qc, s):
                # q/k projection slot s for query-chunk qc
                dstt, col, M = [(qA, 0, 128), (kB, 128, 128),
                                (qC, 256, 64), (kD, 320, 64)][s]
                ps = psM.tile([128, CHQ], f32, tag="mp", name="psb")
                for d in range(ND):
                    nc.tensor.matmul(
                        ps[0:M, :], wqk_r[d][:, col:col + M],
                        xTa[:, 2048 * d + CHQ * qc:2048 * d + CHQ * (qc + 1)],
                        start=(d == 0), stop=(d == ND - 1))
                nc.vector.tensor_scalar_add(
                    dstt[0:M, CHQ * qc:CHQ * (qc + 1)], ps[0:M, :],
                    bias_qk[0:M, s:s + 1])

            def setup_unit_V(t):
                # v for token-tile t in natural [token, 3*64] layout
                psv = psM.tile([128, 192], f32, tag="mp", name="psv")
                for d in range(ND):
                    nc.tensor.matmul(
                        psv[:], xTa[:, 2048 * d + KT * t:2048 * d + KT * (t + 1)],
                        wv_r[d][:], start=(d == 0), stop=False)
                nc.tensor.matmul(psv[:], ones_r[:], bvrow_r[:],
                                 start=False, stop=True)
                dst = vnat[t][:].rearrange("p (h c) -> p h c", c=65)[:, :, 0:64]
                nc.vector.tensor_copy(
                    dst, psv[:].rearrange("p (h c) -> p h c", c=64))

            def setup_chunk(qc):
                for t in range(4 * qc, 4 * qc + 4):
                    setup_unit_A(t)
                for s in range(4):
                    setup_unit_B(qc, s)
                for t in range(4 * qc, 4 * qc + 4):
                    setup_unit_V(t)

            def proj_E(qc):
                # output projection for this core's 128-row quarter of chunk qc
                zsb = []
                for kc in range(ND):
                    zt = zsbp.tile([128, 128], bf16, tag=f"zsb{kc}", name=f"zsb{kc}")
                    nc.sync.dma_start(
                        out=zt[:],
                        in_=ag_out[qc][128 * kc:128 * (kc + 1), bass.ds(coff, 128)])
                    zsb.append(zt)
                DC = D_MODEL // 2
                for dc in range(2):
                    ps = psM.tile([128, DC], f32, tag="mp", name="pso")
                    nc.tensor.matmul(ps[:], ones_r[:], bor[:, DC * dc:DC * (dc + 1)],
                                     start=True, stop=False)
                    for kc in range(ND):
                        nc.tensor.matmul(ps[:], zsb[kc][:],
                                         wo_r[kc][:, DC * dc:DC * (dc + 1)],
                                         start=False, stop=(kc == ND - 1))
                    osb = zsbp.tile([128, DC], f32, tag="osb", name="osb")
                    nc.vector.tensor_copy(osb[:], ps[:])
                    nc.sync.dma_start(
                        out=out_d[KT * qc:KT * (qc + 1), DC * dc:DC * (dc + 1)],
                        in_=osb[:])

            setup_chunk(0)
            if DEBUG_TAPS:
                nc.sync.dma_start(out=dbg["dbg_xT"][:], in_=xTa[:, 0:SEQ])
                nc.sync.dma_start(out=dbg["dbg_qA"][:], in_=qA[:])
                nc.sync.dma_start(out=dbg["dbg_vn"][:], in_=vnat[0][:])
                nc.sync.dma_start(out=dbg["dbg_wo"][:], in_=wo_r[0][:])

            for qc in range(NQC):
                nkt = 4 * qc + 4

                def colo(kt, _qc=qc):
                    return (kt - 4 * _qc) * KT if kt >= 4 * _qc else 0

                for pr in range(nkt // 2):
                    k0, k1 = 2 * pr, 2 * pr + 1
                    lo0, lo1 = colo(k0), colo(k1)
                    pps = [psPP.tile([128, 2 * CHQ], f32, tag="pp",
                                     name=f"pp{h}") for h in range(HPC)]
                    # second tile's matmul starts at lo0 (not lo1) so the
                    # exp'd range [lo0:] is fully written; the extra columns
                    # are finite and never read by the z matmul
                    for j, (kt, lo) in enumerate([(k0, lo0), (k1, lo0)]):
                        for h in range(HPC):
                            kT_, qT_, base = hacc[h]
                            nc.tensor.matmul(
                                pps[h][:, CHQ * j + lo:CHQ * (j + 1)],
                                kT_[base:base + 64, KT * kt:KT * (kt + 1)],
                                qT_[base:base + 64, CHQ * qc + lo:CHQ * (qc + 1)],
                                start=True, stop=True)
                    Ps = []
                    for h in range(HPC):
                        P = pstp.tile([128, 2 * CHQ], bf16, tag="P", name="P")
                        if lo0 == 0:
                            nc.scalar.activation(P[:, lo0:], pps[h][:, lo0:],
                                                 FT.Exp, scale=SCALE)
                        else:
                            # [512, 512+lo0) was never written; exp around it
                            nc.scalar.activation(P[:, lo0:CHQ], pps[h][:, lo0:CHQ],
                                                 FT.Exp, scale=SCALE)
                            nc.scalar.activation(P[:, CHQ + lo0:], pps[h][:, CHQ + lo0:],
                                                 FT.Exp, scale=SCALE)
                        Ps.append(P)
                    for h in range(HPC):
                        P = Ps[h]
                        for j, (kt, lo) in enumerate([(k0, lo0), (k1, lo1)]):
                            if kt >= 4 * qc:
                                nc.vector.tensor_mul(
                                    P[:, CHQ * j + lo:CHQ * j + lo + KT],
                                    P[:, CHQ * j + lo:CHQ * j + lo + KT],
                                    tri[:])
                            nc.tensor.matmul(
                                zps[h][0:65, lo:], vnat[kt][:, 65 * h:65 * (h + 1)],
                                P[:, CHQ * j + lo:CHQ * (j + 1)],
                                start=(kt == 0), stop=(kt == nkt - 1))
                        if DEBUG_TAPS and qc == 0 and pr == 0 and h == 0:
                            nc.sync.dma_start(out=dbg["dbg_P"][:], in_=P[:])

                # normalize: zA = z^T * (1/rowsum), bc broadcast reuses zps bank
                for h in range(HPC):
                    s65 = nrmp.tile([65, CHQ], f32, tag=f"s65_{h}", name="s65")
                    nc.vector.tensor_copy(s65[:], zps[h][0:65, :])
                    rc65 = nrmp.tile([65, CHQ], f32, tag=f"rc_{h}", name="rc65")
                    sc65 = nrmp.tile([65, CHQ], f32, tag=f"sc_{h}", name="sc65")
                    nc.vector.reciprocal_approx_accurate(
                        out=rc65[:], in_=s65[:], scratch=sc65[:])
                    rcb = nrmp.tile([1, CHQ], bf16, tag=f"rcb_{h}", name="rcb")
                    nc.vector.tensor_copy(rcb[:], rc65[64:65, :])
                    nc.tensor.matmul(zps[h][:, :], ones_r[:], rcb[:],
                                     start=True, stop=True)
                    zAc = nrmp.tile([64, CHQ], bf16, tag=f"zA_{h}", name="zAc")
                    nc.vector.tensor_mul(zAc[:], s65[0:64, :], zps[h][0:64, :])
                    nc.sync.dma_start(out=ag_in[qc][64 * h:64 * (h + 1), :],
                                      in_=zAc[:])
                    if DEBUG_TAPS and qc == 0 and h == 0:
                        nc.sync.dma_start(out=dbg["dbg_s65"][0:65, :], in_=s65[:])
                        nc.sync.dma_start(out=dbg["dbg_rcb"][:], in_=rcb[:])
                        nc.sync.dma_start(out=dbg["dbg_zA"][:], in_=zAc[:])

                nc.gpsimd.collective_compute(
                    "AllGather", mybir.AluOpType.bypass,
                    replica_groups=[[0, 1, 2, 3], [4, 5, 6, 7]],
                    ins=[ag_in[qc].opt()], outs=[ag_out[qc].opt()])
                if DEBUG_TAPS and qc == 0:
                    nc.sync.dma_start(out=dbg["dbg_ago"][:], in_=ag_out[qc][:])

                if qc + 1 < NQC:
                    setup_chunk(qc + 1)
                if qc >= 1:
                    proj_E(qc - 1)
            proj_E(NQC - 1)

    nc.compile()
    return nc


def _get_nc():
    global _BUILT
    if _BUILT is None:
        _BUILT = _build()
    return _BUILT


def _make_in_maps(inputs):
    f = np.float32
    x = np.ascontiguousarray(np.asarray(inputs["normalized_resid_pre"], dtype=f))
    W = {"q": np.asarray(inputs["W_Q"], f), "k": np.asarray(inputs["W_K"], f),
         "v": np.asarray(inputs["W_V"], f)}
    Mm = {"q": np.asarray(inputs["mask_W_Q"], f), "k": np.asarray(inputs["mask_W_K"], f),
          "v": np.asarray(inputs["mask_W_V"], f)}
    B = {"q": np.asarray(inputs["b_Q"], f), "k": np.asarray(inputs["b_K"], f),
         "v": np.asarray(inputs["b_V"], f)}
    wo = np.ascontiguousarray(np.asarray(inputs["W_O"], f).reshape(N_HEADS * D_HEAD, D_MODEL))
    mo = np.ascontiguousarray(np.asarray(inputs["mask_W_O"], f).reshape(N_HEADS * D_HEAD, D_MODEL))
    bo = np.asarray(inputs["b_O"], f).reshape(1, D_MODEL)

    in_maps = []
    for c in range(N_CORES):
        b, g = divmod(c, GROUP)
        heads = [HPC * g + i for i in range(HPC)]
        # slots: [q0 q1 k0 k1 q2 k2], 64 cols each
        order = [("q", 0), ("q", 1), ("k", 0), ("k", 1), ("q", 2), ("k", 2)]
        wqk = np.zeros((D_MODEL, 384), f)
        mqk = np.zeros((D_MODEL, 384), f)
        bqk = np.zeros((128, 4), f)
        for s, (mat, hh) in enumerate(order):
            gh = heads[hh]
            wqk[:, 64 * s:64 * (s + 1)] = W[mat][gh]
            mqk[:, 64 * s:64 * (s + 1)] = Mm[mat][gh]
        # bias per proj slot: A=[bq0;bq1], B=[bk0;bk1], C=[bq2;0], D=[bk2;0]
        bqk[0:64, 0] = B["q"][heads[0]]
        bqk[64:128, 0] = B["q"][heads[1]]
        bqk[0:64, 1] = B["k"][heads[0]]
        bqk[64:128, 1] = B["k"][heads[1]]
        bqk[0:64, 2] = B["q"][heads[2]]
        bqk[0:64, 3] = B["k"][heads[2]]
        wv = np.zeros((D_MODEL, 192), f)
        mv = np.zeros((D_MODEL, 192), f)
        bv = np.zeros((1, 192), f)
        for hh in range(HPC):
            gh = heads[hh]
            wv[:, 64 * hh:64 * (hh + 1)] = W["v"][gh]
            mv[:, 64 * hh:64 * (hh + 1)] = Mm["v"][gh]
            bv[0, 64 * hh:64 * (hh + 1)] = B["v"][gh]
        in_maps.append({
            "x": np.ascontiguousarray(x[b]),
            "wqk": wqk, "mqk": mqk, "wv": wv, "mv": mv,
            "wo": wo, "mo": mo,
            "bqk": bqk, "bv": bv, "bo": bo,
        })
    return in_maps


def _run(inputs, trace=False):
    from concourse.bass_utils import run_bass_kernel_spmd
    nc = _get_nc()
    res = run_bass_kernel_spmd(nc, _make_in_maps(inputs),
                               core_ids=list(range(N_CORES)), trace=trace)
    out = np.empty((BATCH, SEQ, D_MODEL), np.float32)
    for c in range(N_CORES):
        b, r = divmod(c, GROUP)
        o = res.results[c]["out"]  # [512, 768]: row block qc -> chunk qc
        for qc in range(NQC):
            out[b, CHQ * qc + KT * r:CHQ * qc + KT * (r + 1), :] = \
                o[KT * qc:KT * (qc + 1), :]
    return out, res


def kernel(**inputs):
    out, _ = _run(inputs, trace=False)
    return out


# revision 18
# speedup vs baseline: 1.0976x; 1.0976x over previous
"""Distributed causal multi-head attention for 8 TRN2 NeuronCores.

Sharding: tensor parallel over heads (3 per core), data parallel over batch
(cores 0-3 = batch 0, cores 4-7 = batch 1).  The kernel is software-pipelined
over four 512-query chunks: transposes and q/k/v projections for later chunks
are interleaved with attention for the current chunk, so the PE array stays
dense while ScalarE streams the softmax exps.  v is produced directly in
natural [token, head*64] layout (no re-transpose).  After each chunk's
attention the normalized z^T head-slabs are exchanged with a 4-core AllToAll
(each core receives only its own 128-query quarter), overlapped with the next
chunk's compute; every core then runs the output projection for its quarter,
so the serial tail is the last small AllToAll plus one projection slab.
"""
import sys
import math
import numpy as np

sys.path.insert(0, "/opt/trn_rl_repo")

D_MODEL, N_HEADS, D_HEAD = 768, 12, 64
BATCH, SEQ = 2, 2048
HPC = 3              # heads per core
GROUP = 4            # cores per batch group
N_CORES = 8
CHQ = 512            # query-chunk width
KT = 128             # key-tile height
NQC = SEQ // CHQ     # 4
NT = SEQ // KT       # 16 token tiles
ND = D_MODEL // 128  # 6 contraction chunks
SCALE = 1.0 / math.sqrt(D_HEAD)

_BUILT = None
DEBUG_TAPS = False


def _build():
    import concourse.bass as bass
    import concourse.bacc as bacc
    import concourse.mybir as mybir
    import concourse.tile as tile
    from concourse.masks import make_identity

    f32 = mybir.dt.float32
    bf16 = mybir.dt.bfloat16
    FT = mybir.ActivationFunctionType

    nc = bacc.Bacc("TRN2", target_bir_lowering=False, debug=False,
                   num_devices=N_CORES)

    x_d = nc.dram_tensor("x", [SEQ, D_MODEL], f32, kind="ExternalInput")
    wqk_d = nc.dram_tensor("wqk", [D_MODEL, 384], f32, kind="ExternalInput")
    mqk_d = nc.dram_tensor("mqk", [D_MODEL, 384], f32, kind="ExternalInput")
    wv_d = nc.dram_tensor("wv", [D_MODEL, 192], f32, kind="ExternalInput")
    mv_d = nc.dram_tensor("mv", [D_MODEL, 192], f32, kind="ExternalInput")
    wo_d = nc.dram_tensor("wo", [N_HEADS * D_HEAD, D_MODEL], f32, kind="ExternalInput")
    mo_d = nc.dram_tensor("mo", [N_HEADS * D_HEAD, D_MODEL], f32, kind="ExternalInput")
    bqk_d = nc.dram_tensor("bqk", [128, 4], f32, kind="ExternalInput")
    bv_d = nc.dram_tensor("bv", [1, 192], f32, kind="ExternalInput")
    bo_d = nc.dram_tensor("bo", [1, D_MODEL], f32, kind="ExternalInput")
    out_d = nc.dram_tensor("out", [CHQ, D_MODEL], f32, kind="ExternalOutput")

    with tile.TileContext(nc) as tc:
        with tc.tile_pool(name="const", bufs=1) as constp, \
             tc.tile_pool(name="dram", bufs=1, space="DRAM") as dramp, \
             tc.tile_pool(name="wt", bufs=1) as wtp, \
             tc.tile_pool(name="big", bufs=1) as bigp, \
             tc.tile_pool(name="wld", bufs=2) as wldp, \
             tc.tile_pool(name="xin", bufs=4) as xinp, \
             tc.tile_pool(name="pst", bufs=4) as pstp, \
             tc.tile_pool(name="nrm", bufs=2) as nrmp, \
             tc.tile_pool(name="zsb", bufs=2) as zsbp, \
             tc.tile_pool(name="psPP", bufs=2, space="PSUM") as psPP, \
             tc.tile_pool(name="psZ", bufs=1, space="PSUM") as psZ, \
             tc.tile_pool(name="psM", bufs=1, space="PSUM") as psM:

            # ---- constants ----
            ident32 = constp.tile([128, 128], f32, tag="id32")
            make_identity(nc, ident32[:])
            ident_r = constp.tile([128, 128], bf16, tag="idr")
            nc.vector.tensor_copy(ident_r[:], ident32[:])
            # tri[p, f] = 1.0 if f >= p else 0.0 (key p visible to query f)
            tri32 = constp.tile([KT, KT], f32, tag="tri32")
            nc.gpsimd.memset(tri32[:], 1.0)
            nc.gpsimd.affine_select(
                out=tri32[:], in_=tri32[:], compare_op=mybir.AluOpType.is_ge,
                fill=0.0, base=0, channel_multiplier=-1, pattern=[[1, KT]])
            tri = constp.tile([KT, KT], bf16, tag="tri")
            nc.vector.tensor_copy(tri[:], tri32[:])
            ones3 = constp.tile([128, HPC], f32, tag="ones3")
            nc.vector.memset(ones3[:], 1.0)
            ones1 = constp.tile([1, 128], f32, tag="ones1")
            nc.vector.memset(ones1[:], 1.0)
            ones_r = constp.tile([1, 128], bf16, tag="ones_r")
            nc.vector.tensor_copy(ones_r[:], ones1[:])
            # preload the exp table set off the critical path
            warm1 = constp.tile([1, 128], f32, tag="warm1")
            nc.scalar.activation(warm1[:], ones1[:], FT.Exp, scale=0.1)
            bias_qk = constp.tile([128, 4], f32, tag="bias_qk")
            nc.gpsimd.dma_start(out=bias_qk[:], in_=bqk_d[:])
            bvrow = constp.tile([1, 192], f32, tag="bvrow")
            nc.gpsimd.dma_start(out=bvrow[:], in_=bv_d[:])
            bvrow_r = constp.tile([1, 192], bf16, tag="bvrow_r")
            nc.vector.tensor_copy(bvrow_r[:], bvrow[:])
            bo32 = constp.tile([1, D_MODEL], f32, tag="bo32")
            nc.gpsimd.dma_start(out=bo32[:], in_=bo_d[:])
            bor = constp.tile([1, D_MODEL], bf16, tag="bor")
            nc.vector.tensor_copy(bor[:], bo32[:])

            # ---- persistent SBUF tensors ----
            xTa = bigp.tile([128, ND * SEQ], bf16, tag="xTa")
            qA = bigp.tile([128, SEQ], bf16, tag="qA")   # [q0 | q1]
            kB = bigp.tile([128, SEQ], bf16, tag="kB")   # [k0 | k1]
            qC = bigp.tile([128, SEQ], bf16, tag="qC")   # [q2 | - ]
            kD = bigp.tile([128, SEQ], bf16, tag="kD")   # [k2 | - ]
            vnat = [bigp.tile([128, 65 * HPC], bf16, tag=f"vn{t}", name=f"vn{t}")
                    for t in range(NT)]
            for t in range(NT):
                vv = vnat[t][:].rearrange("p (h c) -> p h c", c=65)
                nc.vector.tensor_copy(vv[:, :, 64], ones3[:])

            # PE warm-up: harmless matmuls while DMAs stream in
            zps = [psZ.tile([128, CHQ], f32, tag=f"zps{h}", name=f"zps{h}")
                   for h in range(HPC)]
            for _ in range(12):
                nc.tensor.matmul(zps[0][:, 0:128], ident_r[:],
                                 ident_r[:], start=True, stop=True)

            # ---- bulk DMAs: x on sync queue, weights on scalar queue ----
            xr = []
            for t in range(NT):
                xrt = xinp.tile([128, D_MODEL], f32, tag="xr", name="xr", bufs=NT)
                nc.sync.dma_start(out=xrt[:], in_=x_d[KT * t:KT * (t + 1), :])
                xr.append(xrt)

            wqk_r = [wtp.tile([128, 384], bf16, tag=f"wqk{d}", name=f"wqk{d}")
                     for d in range(ND)]
            wv_r = [wtp.tile([128, 192], bf16, tag=f"wv{d}", name=f"wv{d}")
                    for d in range(ND)]
            wo_r = [wtp.tile([128, D_MODEL], bf16, tag=f"wo{d}", name=f"wo{d}")
                    for d in range(ND)]
            for d in range(ND):
                w32 = wldp.tile([128, D_MODEL], f32, tag="w32")
                m32 = wldp.tile([128, D_MODEL], f32, tag="m32")
                nc.scalar.dma_start(out=w32[:, 0:384], in_=wqk_d[128 * d:128 * (d + 1), :])
                nc.scalar.dma_start(out=m32[:, 0:384], in_=mqk_d[128 * d:128 * (d + 1), :])
                nc.scalar.dma_start(out=w32[:, 384:576], in_=wv_d[128 * d:128 * (d + 1), :])
                nc.scalar.dma_start(out=m32[:, 384:576], in_=mv_d[128 * d:128 * (d + 1), :])
                nc.vector.tensor_mul(wqk_r[d][:], w32[:, 0:384], m32[:, 0:384])
                nc.vector.tensor_mul(wv_r[d][:], w32[:, 384:576], m32[:, 384:576])
            for d in range(ND):
                w32 = wldp.tile([128, D_MODEL], f32, tag="w32")
                m32 = wldp.tile([128, D_MODEL], f32, tag="m32")
                nc.scalar.dma_start(out=w32[:], in_=wo_d[128 * d:128 * (d + 1), :])
                nc.scalar.dma_start(out=m32[:], in_=mo_d[128 * d:128 * (d + 1), :])
                nc.gpsimd.tensor_mul(wo_r[d][:], w32[:], m32[:])

            # DRAM collective buffers (per q-chunk): 8-rank AllToAll of
            # query-quarters (quarter d%4 to global rank d; cross-batch
            # halves are ignored by the receiver)
            a2_in = [dramp.tile([N_CORES, HPC * 64, KT], bf16, tag=f"a2i{qc}",
                                name=f"a2i{qc}") for qc in range(NQC)]
            a2_out = [dramp.tile([N_CORES, HPC * 64, KT], bf16, tag=f"a2o{qc}",
                                 name=f"a2o{qc}") for qc in range(NQC)]
            rank = nc.sync.partition_id()
            roff = (rank // GROUP) * (GROUP * HPC * 64)

            dbg = {}
            if DEBUG_TAPS:
                for name, shape, dt in [("dbg_vn", [128, 195], bf16),
                                        ("dbg_s65", [128, CHQ], f32),
                                        ("dbg_zA", [64, CHQ], bf16)]:
                    dbg[name] = nc.dram_tensor(name, shape, dt, kind="ExternalOutput")

            # scores head accessors: (kT tile, qT tile, base partition)
            hacc = [(kB, qA, 0), (kB, qA, 64), (kD, qC, 0)]

            def unit_A(t):
                # cast + transpose x token-tile t into xTa
                xb = xinp.tile([128, D_MODEL], bf16, tag="xb", name="xb")
                nc.vector.tensor_copy(xb[:], xr[t][:])
                pt = psPP.tile([128, D_MODEL], bf16, tag="pp", name="pt")
                for d in range(ND):
                    nc.tensor.transpose(pt[:, 128 * d:128 * (d + 1)],
                                        xb[:, 128 * d:128 * (d + 1)], ident_r[:])
                dst = xTa[:].rearrange("p (d c) -> p d c", d=ND)[:, :, KT * t:KT * (t + 1)]
                src = pt[:].rearrange("p (d c) -> p d c", d=ND)
                nc.vector.tensor_copy(dst, src)

            def unit_B(qc, s):
                # q/k projection slot s for query-chunk qc
                dstt, col, M = [(qA, 0, 128), (kB, 128, 128),
                                (qC, 256, 64), (kD, 320, 64)][s]
                ps = psM.tile([128, CHQ], f32, tag="mp", name="psb")
                for d in range(ND):
                    nc.tensor.matmul(
                        ps[0:M, :], wqk_r[d][:, col:col + M],
                        xTa[:, 2048 * d + CHQ * qc:2048 * d + CHQ * (qc + 1)],
                        start=(d == 0), stop=(d == ND - 1))
                if M == 128:
                    nc.scalar.activation(
                        dstt[0:M, CHQ * qc:CHQ * (qc + 1)], ps[0:M, :],
                        FT.Identity, bias=bias_qk[0:M, s:s + 1])
                else:
                    nc.vector.tensor_scalar_add(
                        dstt[0:M, CHQ * qc:CHQ * (qc + 1)], ps[0:M, :],
                        bias_qk[0:M, s:s + 1])

            def unit_V(t):
                # v for token-tile t in natural [token, 3*64] layout
                psv = psM.tile([128, 192], f32, tag="mp", name="psv")
                for d in range(ND):
                    nc.tensor.matmul(
                        psv[:], xTa[:, 2048 * d + KT * t:2048 * d + KT * (t + 1)],
                        wv_r[d][:], start=(d == 0), stop=False)
                nc.tensor.matmul(psv[:], ones_r[:], bvrow_r[:],
                                 start=False, stop=True)
                dst = vnat[t][:].rearrange("p (h c) -> p h c", c=65)[:, :, 0:64]
                nc.vector.tensor_copy(
                    dst, psv[:].rearrange("p (h c) -> p h c", c=64))
                if DEBUG_TAPS and t == 0:
                    nc.sync.dma_start(out=dbg["dbg_vn"][:], in_=vnat[0][:])

            def proj_E(qc):
                # output projection for this core's 128-row quarter of chunk qc
                aflat = a2_out[qc][:].rearrange("a b c -> (a b) c")
                zsb = []
                for kc in range(ND):
                    zt = zsbp.tile([128, KT], bf16, tag=f"zsb{kc}", name=f"zsb{kc}")
                    nc.sync.dma_start(
                        out=zt[:],
                        in_=aflat[bass.ds(roff + 128 * kc, 128), :])
                    zsb.append(zt)
                DC = D_MODEL // 2
                for dc in range(2):
                    ps = psM.tile([128, DC], f32, tag="mp", name="pso")
                    nc.tensor.matmul(ps[:], ones_r[:], bor[:, DC * dc:DC * (dc + 1)],
                                     start=True, stop=False)
                    for kc in range(ND):
                        nc.tensor.matmul(ps[:], zsb[kc][:],
                                         wo_r[kc][:, DC * dc:DC * (dc + 1)],
                                         start=False, stop=(kc == ND - 1))
                    osb = zsbp.tile([128, DC], f32, tag="osb", name="osb")
                    nc.vector.tensor_copy(osb[:], ps[:])
                    nc.sync.dma_start(
                        out=out_d[KT * qc:KT * (qc + 1), DC * dc:DC * (dc + 1)],
                        in_=osb[:])

            # setup for chunk 0
            for t in range(4):
                unit_A(t)
            for s in range(4):
                unit_B(0, s)
            for t in range(4):
                unit_V(t)

            # A-units for chunks 1-3, doled out between attention pairs
            a_queue = list(range(4, NT))

            for qc in range(NQC):
                nkt = 4 * qc + 4

                def colo(kt, _qc=qc):
                    return (kt - 4 * _qc) * KT if kt >= 4 * _qc else 0

                for pr in range(nkt // 2):
                    k0, k1 = 2 * pr, 2 * pr + 1
                    lo0, lo1 = colo(k0), colo(k1)
                    pps = [psPP.tile([128, 2 * CHQ], f32, tag="pp",
                                     name=f"pp{h}") for h in range(HPC)]
                    # second tile's matmul starts at lo0 (not lo1) so the
                    # exp'd range [lo0:] is fully written; the extra columns
                    # are finite and never read by the z matmul
                    for j, (kt, lo) in enumerate([(k0, lo0), (k1, lo0)]):
                        for h in range(HPC):
                            kT_, qT_, base = hacc[h]
                            nc.tensor.matmul(
                                pps[h][:, CHQ * j + lo:CHQ * (j + 1)],
                                kT_[base:base + 64, KT * kt:KT * (kt + 1)],
                                qT_[base:base + 64, CHQ * qc + lo:CHQ * (qc + 1)],
                                start=True, stop=True)
                    Ps = []
                    for h in range(HPC):
                        P = pstp.tile([128, 2 * CHQ], bf16, tag="P", name="P")
                        if lo0 == 0:
                            nc.scalar.activation(P[:, lo0:], pps[h][:, lo0:],
                                                 FT.Exp, scale=SCALE)
                        else:
                            # [512, 512+lo0) was never written; exp around it
                            nc.scalar.activation(P[:, lo0:CHQ], pps[h][:, lo0:CHQ],
                                                 FT.Exp, scale=SCALE)
                            nc.scalar.activation(P[:, CHQ + lo0:], pps[h][:, CHQ + lo0:],
                                                 FT.Exp, scale=SCALE)
                        Ps.append(P)
                    for h in range(HPC):
                        P = Ps[h]
                        for j, (kt, lo) in enumerate([(k0, lo0), (k1, lo1)]):
                            if kt >= 4 * qc:
                                nc.vector.tensor_mul(
                                    P[:, CHQ * j + lo:CHQ * j + lo + KT],
                                    P[:, CHQ * j + lo:CHQ * j + lo + KT],
                                    tri[:])
                            nc.tensor.matmul(
                                zps[h][0:65, lo:], vnat[kt][:, 65 * h:65 * (h + 1)],
                                P[:, CHQ * j + lo:CHQ * (j + 1)],
                                start=(kt == 0), stop=(kt == nkt - 1))
                    # keep the transpose stream flowing between pairs
                    for _ in range(2):
                        if a_queue:
                            unit_A(a_queue.pop(0))

                # normalize: zA = z^T * (1/rowsum), bc broadcast reuses zps bank
                for h in range(HPC):
                    s65 = nrmp.tile([65, CHQ], f32, tag=f"s65_{h}", name="s65")
                    nc.vector.tensor_copy(s65[:], zps[h][0:65, :])
                    rc65 = nrmp.tile([65, CHQ], f32, tag=f"rc_{h}", name="rc65")
                    nc.vector.reciprocal_approx_fast(out=rc65[:], in_=s65[:])
                    rcb = nrmp.tile([1, CHQ], bf16, tag=f"rcb_{h}", name="rcb")
                    nc.vector.tensor_copy(rcb[:], rc65[64:65, :])
                    nc.tensor.matmul(zps[h][:, :], ones_r[:], rcb[:],
                                     start=True, stop=True)
                    zAc = nrmp.tile([64, CHQ], bf16, tag=f"zA_{h}", name="zAc")
                    nc.vector.tensor_mul(zAc[:], s65[0:64, :], zps[h][0:64, :])
                    for d in range(N_CORES):
                        nc.sync.dma_start(
                            out=a2_in[qc][d, 64 * h:64 * (h + 1), :],
                            in_=zAc[:, KT * (d % GROUP):KT * (d % GROUP + 1)])
                    if DEBUG_TAPS and qc == 0 and h == 0:
                        nc.sync.dma_start(out=dbg["dbg_s65"][0:65, :], in_=s65[:])
                        nc.sync.dma_start(out=dbg["dbg_zA"][:], in_=zAc[:])

                nc.gpsimd.collective_compute(
                    "AllToAll", mybir.AluOpType.bypass,
                    replica_groups=[list(range(N_CORES))],
                    ins=[a2_in[qc].opt()], outs=[a2_out[qc].opt()])

                if qc >= 1:
                    proj_E(qc - 1)
                if qc + 1 < NQC:
                    for s in range(4):
                        unit_B(qc + 1, s)
                    for t in range(4 * qc + 4, 4 * qc + 8):
                        unit_V(t)
            proj_E(NQC - 1)

    nc.compile()
    return nc


def _get_nc():
    global _BUILT
    if _BUILT is None:
        _BUILT = _build()
    return _BUILT


def _make_in_maps(inputs):
    f = np.float32
    x = np.ascontiguousarray(np.asarray(inputs["normalized_resid_pre"], dtype=f))
    W = {"q": np.asarray(inputs["W_Q"], f), "k": np.asarray(inputs["W_K"], f),
         "v": np.asarray(inputs["W_V"], f)}
    Mm = {"q": np.asarray(inputs["mask_W_Q"], f), "k": np.asarray(inputs["mask_W_K"], f),
          "v": np.asarray(inputs["mask_W_V"], f)}
    B = {"q": np.asarray(inputs["b_Q"], f), "k": np.asarray(inputs["b_K"], f),
         "v": np.asarray(inputs["b_V"], f)}
    wo = np.ascontiguousarray(np.asarray(inputs["W_O"], f).reshape(N_HEADS * D_HEAD, D_MODEL))
    mo = np.ascontiguousarray(np.asarray(inputs["mask_W_O"], f).reshape(N_HEADS * D_HEAD, D_MODEL))
    bo = np.asarray(inputs["b_O"], f).reshape(1, D_MODEL)

    in_maps = []
    for c in range(N_CORES):
        b, g = divmod(c, GROUP)
        heads = [HPC * g + i for i in range(HPC)]
        # slots: [q0 q1 k0 k1 q2 k2], 64 cols each
        order = [("q", 0), ("q", 1), ("k", 0), ("k", 1), ("q", 2), ("k", 2)]
        wqk = np.zeros((D_MODEL, 384), f)
        mqk = np.zeros((D_MODEL, 384), f)
        bqk = np.zeros((128, 4), f)
        for s, (mat, hh) in enumerate(order):
            gh = heads[hh]
            wqk[:, 64 * s:64 * (s + 1)] = W[mat][gh]
            mqk[:, 64 * s:64 * (s + 1)] = Mm[mat][gh]
        bqk[0:64, 0] = B["q"][heads[0]]
        bqk[64:128, 0] = B["q"][heads[1]]
        bqk[0:64, 1] = B["k"][heads[0]]
        bqk[64:128, 1] = B["k"][heads[1]]
        bqk[0:64, 2] = B["q"][heads[2]]
        bqk[0:64, 3] = B["k"][heads[2]]
        wv = np.zeros((D_MODEL, 192), f)
        mv = np.zeros((D_MODEL, 192), f)
        bv = np.zeros((1, 192), f)
        for hh in range(HPC):
            gh = heads[hh]
            wv[:, 64 * hh:64 * (hh + 1)] = W["v"][gh]
            mv[:, 64 * hh:64 * (hh + 1)] = Mm["v"][gh]
            bv[0, 64 * hh:64 * (hh + 1)] = B["v"][gh]
        in_maps.append({
            "x": np.ascontiguousarray(x[b]),
            "wqk": wqk, "mqk": mqk, "wv": wv, "mv": mv,
            "wo": wo, "mo": mo,
            "bqk": bqk, "bv": bv, "bo": bo,
        })
    return in_maps


def _run(inputs, trace=False):
    from concourse.bass_utils import run_bass_kernel_spmd
    nc = _get_nc()
    res = run_bass_kernel_spmd(nc, _make_in_maps(inputs),
                               core_ids=list(range(N_CORES)), trace=trace)
    out = np.empty((BATCH, SEQ, D_MODEL), np.float32)
    for c in range(N_CORES):
        b, r = divmod(c, GROUP)
        o = res.results[c]["out"]  # [512, 768]: row block qc -> chunk qc
        for qc in range(NQC):
            out[b, CHQ * qc + KT * r:CHQ * qc + KT * (r + 1), :] = \
                o[KT * qc:KT * (qc + 1), :]
    return out, res


def kernel(**inputs):
    out, _ = _run(inputs, trace=False)
    return out
